# revision 14
# baseline (speedup 1.0000x reference)
"""Trainium2 Bass kernel for nn_MultiHeadGraphAttention.

Multi-head graph attention (GAT-style):
    h_prime = einsum('nf,hfo->hno', h, w)
    attn    = softmax(where(adj, leakyrelu(s_i + d_j), -inf), axis=-1)
    out     = attn @ h_prime + b
with s = h_prime @ a_src, d = h_prime @ a_dst, n=4096, H=8, f_out=64.

Sharding: one head per NeuronCore (8 cores). Inside a core everything is
computed in a transposed [j, i] layout so the attention matrix feeds the
PE matmul directly as the moving operand with contraction over j:

  P^T[j, i] = adj[i, j] * exp(leakyrelu(s_i + d_j))
  out^T[o, i] = sum_j Vtilde[j, o] * P^T[j, i]     (Vtilde = [h_prime | 1])

The ones column of Vtilde yields the softmax denominators for free.
exp(leakyrelu(x)) == max(exp(x), exp(0.2 x)) (exp is monotonic), giving
two per-chunk strategies balanced across ScalarE and VectorE:
  - "exp"  : ACT Prelu(alpha=0.2) + ACT Exp (bias = d_j per-partition),
             then one DVE mask multiply
  - "rank1": exp(s_i + d_j) = u_i * v_j is rank-1; DVE-only via
             tensor_scalar on precomputed u = exp(s) broadcast tiles and
             v = exp(d) per-partition columns, then max + mask.

fp32 matmuls run at half rate on TRN2's PE, so the h_prime/s/d
projections use bf16 hi+lo split-compensation (exact to ~fp32 for s/d:
bf16 products are exact in fp32, and the h*wa projections accumulate
hi*hi + hi*lo + lo*hi (+lo*lo) in fp32 PSUM).

Host-side prep (sharding/layout only): h transposed + split hi/lo bf16,
adj transposed and cast to bf16 {0,1}, per-head weight columns, final
out^T -> out transpose.
"""
import sys

if "/opt/trn_rl_repo" not in sys.path:
    sys.path.insert(0, "/opt/trn_rl_repo")

from contextlib import ExitStack

import ml_dtypes
import numpy as np

import concourse.bass as bass
import concourse.bacc as bacc
import concourse.tile as tile
from concourse import mybir
from concourse.bass_utils import run_bass_kernel_spmd

F32 = mybir.dt.float32
BF16 = mybir.dt.bfloat16
AF = mybir.ActivationFunctionType
ALU = mybir.AluOpType

N = 4096
F_IN = 256
N_HEAD = 8
F_OUT = 64
NEG = 0.2
NCH = N // 128        # 32 j-chunks
NSL = N // 512        # 8 512-slices
FC = F_IN // 128      # 2 f-chunks
VW = F_OUT + 1        # 65: V columns + ones column

# Per-chunk route: "exp" (2x ACT + 1x DVE) or "rank1" (4x DVE).
# exp-heavy at the start (rank1 needs the exp(d)/exp(s) tables).
_R1 = {2, 5, 8, 10, 13, 16, 18, 21, 24, 26, 29, 31}
ROUTES = ["rank1" if i in _R1 else "exp" for i in range(NCH)]
assert len(ROUTES) == NCH and sum(r == "exp" for r in ROUTES) == 20


def build_program(routes=ROUTES, use_prelu=True):
    nc = bacc.Bacc("TRN2", target_bir_lowering=False, debug=False)
    hT_hi = nc.dram_tensor("hT_hi", [F_IN, N], BF16, kind="ExternalInput").ap()
    hT_lo = nc.dram_tensor("hT_lo", [F_IN, N], BF16, kind="ExternalInput").ap()
    w_bf = nc.dram_tensor("w_bf", [F_IN, F_OUT], BF16, kind="ExternalInput").ap()
    wa_hi3 = nc.dram_tensor("wa_hi3", [F_IN, 3], BF16, kind="ExternalInput").ap()
    wa_lo3 = nc.dram_tensor("wa_lo3", [F_IN, 3], BF16, kind="ExternalInput").ap()
    eye3 = nc.dram_tensor("eye3", [3, 3], F32, kind="ExternalInput").ap()
    maskT = nc.dram_tensor("maskT", [N, N], BF16, kind="ExternalInput").ap()
    bvec = nc.dram_tensor("bvec", [128, F_OUT], F32, kind="ExternalInput").ap()
    outT = nc.dram_tensor("outT", [F_OUT, N], F32, kind="ExternalOutput").ap()
    s_dram = nc.dram_tensor("s_scratch", [N], F32).ap()
    sv_dram = nc.dram_tensor("sv_scratch", [N], F32).ap()
    r_dram = nc.dram_tensor("r_scratch", [N], F32).ap()

    with tile.TileContext(nc) as tc, ExitStack() as ctx:
        const_pool = ctx.enter_context(tc.tile_pool(name="const", bufs=1))
        p_pool = ctx.enter_context(tc.tile_pool(name="pw", bufs=3))
        t_pool = ctx.enter_context(tc.tile_pool(name="tw", bufs=2))
        e_pool = ctx.enter_context(tc.tile_pool(name="ew", bufs=2))
        c_pool = ctx.enter_context(tc.tile_pool(name="cw", bufs=2))
        mask_pool = ctx.enter_context(tc.tile_pool(name="mask", bufs=2))
        pre_ctx = ExitStack()
        psw_pool = pre_ctx.enter_context(tc.tile_pool(name="psw", bufs=2, space="PSUM"))
        pre_pool = pre_ctx.enter_context(tc.tile_pool(name="pre", bufs=1))

        # ---------------- constant loads ----------------
        hThi_sb = pre_pool.tile([128, FC * N], BF16, tag="hThi")
        hTlo_sb = pre_pool.tile([128, FC * N], BF16, tag="hTlo")
        H2 = N // 2
        for half in range(2):
            for fc in range(FC):
                nc.sync.dma_start(
                    hThi_sb[:, fc * N + half * H2: fc * N + (half + 1) * H2],
                    hT_hi[fc * 128:(fc + 1) * 128, half * H2:(half + 1) * H2])
            for fc in range(FC):
                nc.sync.dma_start(
                    hTlo_sb[:, fc * N + half * H2: fc * N + (half + 1) * H2],
                    hT_lo[fc * 128:(fc + 1) * 128, half * H2:(half + 1) * H2])
        wahi_sb = const_pool.tile([128, FC * 3], BF16, tag="wahi")
        walo_sb = const_pool.tile([128, FC * 3], BF16, tag="walo")
        for fc in range(FC):
            nc.sync.dma_start(wahi_sb[:, fc * 3:(fc + 1) * 3], wa_hi3[fc * 128:(fc + 1) * 128, :])
            nc.sync.dma_start(walo_sb[:, fc * 3:(fc + 1) * 3], wa_lo3[fc * 128:(fc + 1) * 128, :])
        w_sb = const_pool.tile([128, FC * F_OUT], BF16, tag="w")
        for fc in range(FC):
            nc.sync.dma_start(w_sb[:, fc * F_OUT:(fc + 1) * F_OUT],
                              w_bf[fc * 128:(fc + 1) * 128, :])
        eye_sb = const_pool.tile([3, 3], F32, tag="eye")
        nc.sync.dma_start(eye_sb[:, :], eye3[:, :])
        b_sb = const_pool.tile([128, F_OUT], F32, tag="b")
        nc.sync.dma_start(b_sb[:, :], bvec[:, :])

        # ---------------- s^T/d^T rows (hi/lo compensated) ----------------
        # lhsT = wa6 [f, 6] = [s_hi d_hi d5_hi s_lo d_lo d5_lo] coefficient
        # columns; accumulating over {hi,lo} x {f-chunks} of h gives rows
        # 0:3 = (h_hi+h_lo)*hi-coef... true rows = row[k] + row[k+3].
        sdT_sb = pre_pool.tile([3, N], F32, tag="sdT")
        for sl in range(NSL):
            ps_sd = psw_pool.tile([3, 512], F32, tag="pssd")
            combos = [(wa, hsb) for fc in range(FC)
                      for wa in (wahi_sb, walo_sb) for hsb in (hThi_sb, hTlo_sb)]
            ncomb = len(combos)
            for ci, (wa, hsb) in enumerate(combos):
                fc = ci // 4
                nc.tensor.matmul(ps_sd[:, :], wa[:, fc * 3:(fc + 1) * 3],
                                 hsb[:, fc * N + sl * 512: fc * N + (sl + 1) * 512],
                                 start=(ci == 0), stop=(ci == ncomb - 1))
            nc.vector.tensor_copy(sdT_sb[0:3, sl * 512:(sl + 1) * 512], ps_sd[:, :])

        # ---------------- d columns via PE transpose ----------------
        d_sb = const_pool.tile([128, 2 * NCH], F32, tag="d")
        v_sb = const_pool.tile([128, 2 * NCH], F32, tag="v")
        for jc in range(NCH):
            ps_t = psw_pool.tile([128, 3], F32, tag="pst")
            nc.tensor.transpose(ps_t[:, :], sdT_sb[0:3, jc * 128:(jc + 1) * 128],
                                eye_sb[:, :])
            nc.vector.tensor_copy(d_sb[:, 2 * jc: 2 * jc + 2], ps_t[:, 1:3])
        nc.scalar.activation(v_sb[:, :], d_sb[:, :], AF.Exp)

        # ---------------- S broadcast (via DRAM) + exp tables ----------------
        nc.sync.dma_start(s_dram[:], sdT_sb[0:1, :])
        S_b = const_pool.tile([128, N], F32, tag="Sb")
        nc.sync.dma_start(S_b[:, :], s_dram[None, :].broadcast_to((128, N)))
        U_b = const_pool.tile([128, N], BF16, tag="Ub")
        U2_b = const_pool.tile([128, N], BF16, tag="U2b")
        nc.scalar.activation(U_b[:, :], S_b[:, :], AF.Exp)
        nc.scalar.activation(U2_b[:, :], S_b[:, :], AF.Exp, scale=NEG)

        # ---------------- h_prime (V, bf16) ----------------
        V_sb = const_pool.tile([128, NCH * VW], BF16, tag="V")
        nc.vector.memset(V_sb[:, :], 1.0)
        for jc in range(NCH):
            ps_v = psw_pool.tile([128, F_OUT], F32, tag="psv")
            for fc in range(FC):
                nc.tensor.matmul(
                    ps_v[:, :],
                    hThi_sb[:, fc * N + jc * 128: fc * N + (jc + 1) * 128],
                    w_sb[:, fc * F_OUT:(fc + 1) * F_OUT],
                    start=(fc == 0), stop=(fc == FC - 1),
                )
            nc.vector.tensor_tensor(V_sb[:, jc * VW: jc * VW + F_OUT], ps_v[:, :],
                                    b_sb[:, :], op=ALU.add)

        # ---------------- attention j-loop ----------------
        pre_ctx.close()  # release pre-phase PSUM banks + h_T staging
        psbig_pool = ctx.enter_context(tc.tile_pool(name="psbig", bufs=1, space="PSUM"))
        ps_O = psbig_pool.tile([VW, N], F32, tag="psBIG")
        for jc in range(NCH):
            m_t = mask_pool.tile([128, N], BF16, tag="mt")
            nc.scalar.dma_start(m_t[:, :], maskT[jc * 128:(jc + 1) * 128, :])
            p_t = p_pool.tile([128, N], BF16, tag="pt")
            if routes[jc] == "exp":
                if use_prelu:
                    t_t = t_pool.tile([128, N], F32, tag="tt")
                    nc.scalar.activation(t_t[:, :], S_b[:, :], AF.Prelu,
                                         bias=d_sb[:, 2 * jc:2 * jc + 1], alpha=NEG)
                    e1_t = e_pool.tile([128, N], BF16, tag="et")
                    nc.scalar.activation(e1_t[:, :], t_t[:, :], AF.Exp)
                    nc.vector.tensor_tensor(p_t[:, :], e1_t[:, :], m_t[:, :], op=ALU.mult)
                else:
                    e1_t = t_pool.tile([128, N], BF16, tag="tt")
                    nc.scalar.activation(e1_t[:, :], S_b[:, :], AF.Exp,
                                         bias=d_sb[:, 2 * jc:2 * jc + 1])
                    e2_t = e_pool.tile([128, N], BF16, tag="et")
                    nc.scalar.activation(e2_t[:, :], S_b[:, :], AF.Exp, scale=NEG,
                                         bias=d_sb[:, 2 * jc + 1:2 * jc + 2])
                    c_t = c_pool.tile([128, N], BF16, tag="ct")
                    nc.vector.tensor_tensor(c_t[:, :], e1_t[:, :], e2_t[:, :], op=ALU.max)
                    nc.vector.tensor_tensor(p_t[:, :], c_t[:, :], m_t[:, :], op=ALU.mult)
            else:
                a_t = e_pool.tile([128, N], BF16, tag="et")
                nc.vector.tensor_scalar(a_t[:, :], U_b[:, :],
                                        v_sb[:, 2 * jc:2 * jc + 1], None, op0=ALU.mult)
                b_t = c_pool.tile([128, N], BF16, tag="ct")
                nc.vector.tensor_scalar(b_t[:, :], U2_b[:, :],
                                        v_sb[:, 2 * jc + 1:2 * jc + 2], None, op0=ALU.mult)
                c_t = t_pool.tile([128, N], BF16, tag="tt2")
                nc.vector.tensor_tensor(c_t[:, :], a_t[:, :], b_t[:, :], op=ALU.max)
                nc.vector.tensor_tensor(p_t[:, :], c_t[:, :], m_t[:, :], op=ALU.mult)
            for k in range(NSL):
                nc.tensor.matmul(ps_O[:, k * 512:(k + 1) * 512],
                                 V_sb[:, jc * VW:(jc + 1) * VW],
                                 p_t[:, k * 512:(k + 1) * 512],
                                 start=(jc == 0), stop=(jc == NCH - 1))

        # ---------------- tail: normalize + bias (no PE) ----------------
        tail_pool = ctx.enter_context(tc.tile_pool(name="tail", bufs=1))
        O_sb = tail_pool.tile([VW, N], F32, tag="Osb")
        nc.vector.tensor_copy(O_sb[:, :], ps_O[:, :])
        nc.sync.dma_start(sv_dram[:], O_sb[F_OUT:VW, :])
        sres = tail_pool.tile([128, N // 128], F32, tag="sres")
        nc.sync.dma_start(sres[:, :], sv_dram.rearrange("(p q) -> p q", p=128))
        rres = tail_pool.tile([128, N // 128], F32, tag="rres")
        nc.vector.reciprocal(rres[:, :], sres[:, :])
        nc.sync.dma_start(r_dram.rearrange("(p q) -> p q", p=128), rres[:, :])
        R_sb = tail_pool.tile([F_OUT, N], F32, tag="Rsb")
        nc.sync.dma_start(R_sb[:, :], r_dram[None, :].broadcast_to((F_OUT, N)))
        F_sb = tail_pool.tile([F_OUT, N], F32, tag="Fsb")
        nc.vector.tensor_tensor(F_sb[:, :], O_sb[0:F_OUT, :], R_sb[:, :], op=ALU.mult)
        nc.sync.dma_start(outT[:, :], F_sb[:, :])
    nc.compile()
    return nc


_CACHED_NC = None


def _get_nc():
    global _CACHED_NC
    if _CACHED_NC is None:
        _CACHED_NC = build_program()
    return _CACHED_NC


def _split_hilo(x):
    hi = x.astype(ml_dtypes.bfloat16)
    lo = (x - hi.astype(np.float32)).astype(ml_dtypes.bfloat16)
    return hi, lo


def _prep_inputs(h, adj, w, a_src, a_dst, b):
    h = np.asarray(h, dtype=np.float32)
    adj = np.asarray(adj)
    w = np.asarray(w, dtype=np.float32)
    a_src = np.asarray(a_src, dtype=np.float32)
    a_dst = np.asarray(a_dst, dtype=np.float32)
    b = np.asarray(b, dtype=np.float32)

    h_T = np.ascontiguousarray(h.T)
    hT_hi, hT_lo = _split_hilo(h_T)
    maskT = np.ascontiguousarray(adj.T).astype(ml_dtypes.bfloat16)
    b_bc = np.ascontiguousarray(np.broadcast_to(b[None, :], (128, F_OUT)),
                                dtype=np.float32)
    eye = np.eye(3, dtype=np.float32)

    in_maps = []
    for c in range(N_HEAD):
        wa_src = (w[c] @ a_src[c])[:, 0]              # [F_IN]
        wa_dst = (w[c] @ a_dst[c])[:, 0]
        cols = [wa_src, wa_dst, NEG * wa_dst]
        his, los = [], []
        for q in cols:
            qh, ql = _split_hilo(q.astype(np.float32))
            his.append(qh)
            los.append(ql)
        wa_hi3 = np.stack([x.astype(np.float32) for x in his], axis=1).astype(ml_dtypes.bfloat16)
        wa_lo3 = np.stack([x.astype(np.float32) for x in los], axis=1).astype(ml_dtypes.bfloat16)
        in_maps.append({
            "hT_hi": hT_hi,
            "hT_lo": hT_lo,
            "w_bf": np.ascontiguousarray(w[c]).astype(ml_dtypes.bfloat16),
            "wa_hi3": np.ascontiguousarray(wa_hi3),
            "wa_lo3": np.ascontiguousarray(wa_lo3),
            "eye3": eye,
            "maskT": maskT,
            "bvec": b_bc,
        })
    return in_maps


def _run(in_maps, trace=False, **kwargs):
    nc = _get_nc()
    return run_bass_kernel_spmd(nc, in_maps, list(range(N_HEAD)), trace=trace, **kwargs)


def kernel(h, adj, w, a_src, a_dst, b):
    in_maps = _prep_inputs(h, adj, w, a_src, a_dst, b)
    res = _run(in_maps)
    out = np.stack([np.ascontiguousarray(res.results[c]["outT"].T)
                    for c in range(N_HEAD)])
    return out.astype(np.float32)


# revision 15
# speedup vs baseline: 1.1247x; 1.1247x over previous
"""Trainium2 Bass kernel for nn_MultiHeadGraphAttention.

Multi-head graph attention (GAT-style):
    h_prime = einsum('nf,hfo->hno', h, w)
    attn    = softmax(where(adj, leakyrelu(s_i + d_j), -inf), axis=-1)
    out     = attn @ h_prime + b
with s = h_prime @ a_src, d = h_prime @ a_dst, n=4096, H=8, f_out=64.

Sharding: one head per NeuronCore (8 cores). Inside a core everything is
computed in a transposed [j, i] layout so the attention matrix feeds the
PE matmul directly as the moving operand with contraction over j:

  P^T[j, i] = adj[i, j] * exp(leakyrelu(s_i + d_j))
  out^T[o, i] = sum_j Vtilde[j, o] * P^T[j, i]     (Vtilde = [h_prime | 1])

The ones column of Vtilde yields the softmax denominators for free.
exp(leakyrelu(x)) == max(exp(x), exp(0.2 x)) (exp is monotonic), giving
two per-chunk strategies balanced across ScalarE and VectorE:
  - "exp"  : ACT Prelu(alpha=0.2) + ACT Exp (bias = d_j per-partition),
             then one DVE mask multiply
  - "rank1": exp(s_i + d_j) = u_i * v_j is rank-1; DVE-only via
             tensor_scalar on precomputed u = exp(s) broadcast tiles and
             v = exp(d) per-partition columns, then max + mask.

fp32 matmuls run at half rate on TRN2's PE, so the h_prime/s/d
projections use bf16 hi+lo split-compensation (exact to ~fp32 for s/d:
bf16 products are exact in fp32, and the h*wa projections accumulate
hi*hi + hi*lo + lo*hi (+lo*lo) in fp32 PSUM).

Host-side prep (sharding/layout only): h transposed + split hi/lo bf16,
adj transposed and cast to bf16 {0,1}, per-head weight columns, final
out^T -> out transpose.
"""
import sys

if "/opt/trn_rl_repo" not in sys.path:
    sys.path.insert(0, "/opt/trn_rl_repo")

from contextlib import ExitStack

import ml_dtypes
import numpy as np

import concourse.bass as bass
import concourse.bacc as bacc
import concourse.tile as tile
from concourse import mybir
from concourse.bass_utils import run_bass_kernel_spmd

F32 = mybir.dt.float32
BF16 = mybir.dt.bfloat16
AF = mybir.ActivationFunctionType
ALU = mybir.AluOpType

N = 4096
F_IN = 256
N_HEAD = 8
F_OUT = 64
NEG = 0.2
NCH = N // 128        # 32 j-chunks
NSL = N // 512        # 8 512-slices
FC = F_IN // 128      # 2 f-chunks
VW = F_OUT + 1        # 65: V columns + ones column

# Per-chunk route: "exp" (2x ACT + 1x DVE) or "rank1" (4x DVE).
# exp-heavy at the start (rank1 needs the exp(d)/exp(s) tables).
_R1 = {2, 5, 8, 10, 13, 16, 18, 21, 24, 26, 29, 31}
ROUTES = ["rank1" if i in _R1 else "exp" for i in range(NCH)]
assert len(ROUTES) == NCH and sum(r == "exp" for r in ROUTES) == 20


def build_program(routes=ROUTES, use_prelu=True):
    nc = bacc.Bacc("TRN2", target_bir_lowering=False, debug=False)
    hT_hi = nc.dram_tensor("hT_hi", [F_IN, N], BF16, kind="ExternalInput").ap()
    hT_lo = nc.dram_tensor("hT_lo", [F_IN, N], BF16, kind="ExternalInput").ap()
    w_bf = nc.dram_tensor("w_bf", [F_IN, F_OUT], BF16, kind="ExternalInput").ap()
    wa_hi3 = nc.dram_tensor("wa_hi3", [F_IN, 3], BF16, kind="ExternalInput").ap()
    wa_lo3 = nc.dram_tensor("wa_lo3", [F_IN, 3], BF16, kind="ExternalInput").ap()
    eye3 = nc.dram_tensor("eye3", [3, 3], F32, kind="ExternalInput").ap()
    maskT = nc.dram_tensor("maskT", [N, N], BF16, kind="ExternalInput").ap()
    bvec = nc.dram_tensor("bvec", [128, F_OUT], F32, kind="ExternalInput").ap()
    outT = nc.dram_tensor("outT", [F_OUT, N], F32, kind="ExternalOutput").ap()
    s_dram = nc.dram_tensor("s_scratch", [N], F32).ap()
    sv_dram = nc.dram_tensor("sv_scratch", [N], F32).ap()
    r_dram = nc.dram_tensor("r_scratch", [N], F32).ap()

    with tile.TileContext(nc) as tc, ExitStack() as ctx:
        const_pool = ctx.enter_context(tc.tile_pool(name="const", bufs=1))
        p_pool = ctx.enter_context(tc.tile_pool(name="pw", bufs=3))
        t_pool = ctx.enter_context(tc.tile_pool(name="tw", bufs=2))
        e_pool = ctx.enter_context(tc.tile_pool(name="ew", bufs=2))
        c_pool = ctx.enter_context(tc.tile_pool(name="cw", bufs=2))
        mask_pool = ctx.enter_context(tc.tile_pool(name="mask", bufs=2))
        pre_ctx = ExitStack()
        psw_pool = pre_ctx.enter_context(tc.tile_pool(name="psw", bufs=2, space="PSUM"))
        pre_pool = pre_ctx.enter_context(tc.tile_pool(name="pre", bufs=1))

        # ---------------- constant loads ----------------
        hThi_sb = pre_pool.tile([128, FC * N], BF16, tag="hThi")
        hTlo_sb = pre_pool.tile([128, FC * N], BF16, tag="hTlo")
        H2 = N // 2
        for half in range(2):
            for fc in range(FC):
                nc.sync.dma_start(
                    hThi_sb[:, fc * N + half * H2: fc * N + (half + 1) * H2],
                    hT_hi[fc * 128:(fc + 1) * 128, half * H2:(half + 1) * H2])
            for fc in range(FC):
                nc.sync.dma_start(
                    hTlo_sb[:, fc * N + half * H2: fc * N + (half + 1) * H2],
                    hT_lo[fc * 128:(fc + 1) * 128, half * H2:(half + 1) * H2])
        wahi_sb = const_pool.tile([128, FC * 3], BF16, tag="wahi")
        walo_sb = const_pool.tile([128, FC * 3], BF16, tag="walo")
        for fc in range(FC):
            nc.sync.dma_start(wahi_sb[:, fc * 3:(fc + 1) * 3], wa_hi3[fc * 128:(fc + 1) * 128, :])
            nc.sync.dma_start(walo_sb[:, fc * 3:(fc + 1) * 3], wa_lo3[fc * 128:(fc + 1) * 128, :])
        w_sb = const_pool.tile([128, FC * F_OUT], BF16, tag="w")
        for fc in range(FC):
            nc.sync.dma_start(w_sb[:, fc * F_OUT:(fc + 1) * F_OUT],
                              w_bf[fc * 128:(fc + 1) * 128, :])
        eye_sb = const_pool.tile([3, 3], F32, tag="eye")
        nc.sync.dma_start(eye_sb[:, :], eye3[:, :])
        b_sb = const_pool.tile([128, F_OUT], F32, tag="b")
        nc.sync.dma_start(b_sb[:, :], bvec[:, :])

        # ---------------- s^T/d^T rows (hi/lo compensated) ----------------
        # lhsT = wa6 [f, 6] = [s_hi d_hi d5_hi s_lo d_lo d5_lo] coefficient
        # columns; accumulating over {hi,lo} x {f-chunks} of h gives rows
        # 0:3 = (h_hi+h_lo)*hi-coef... true rows = row[k] + row[k+3].
        sdT_sb = pre_pool.tile([3, N], F32, tag="sdT")
        for sl in range(NSL):
            ps_sd = psw_pool.tile([3, 512], F32, tag="pssd")
            combos = [(wa, hsb) for fc in range(FC)
                      for wa in (wahi_sb, walo_sb) for hsb in (hThi_sb, hTlo_sb)]
            ncomb = len(combos)
            for ci, (wa, hsb) in enumerate(combos):
                fc = ci // 4
                nc.tensor.matmul(ps_sd[:, :], wa[:, fc * 3:(fc + 1) * 3],
                                 hsb[:, fc * N + sl * 512: fc * N + (sl + 1) * 512],
                                 start=(ci == 0), stop=(ci == ncomb - 1))
            nc.vector.tensor_copy(sdT_sb[0:3, sl * 512:(sl + 1) * 512], ps_sd[:, :])

        # ---------------- d columns via PE transpose ----------------
        d_sb = const_pool.tile([128, 2 * NCH], F32, tag="d")
        v_sb = const_pool.tile([128, 2 * NCH], F32, tag="v")
        for jc in range(NCH):
            ps_t = psw_pool.tile([128, 3], F32, tag="pst")
            nc.tensor.transpose(ps_t[:, :], sdT_sb[0:3, jc * 128:(jc + 1) * 128],
                                eye_sb[:, :])
            nc.vector.tensor_copy(d_sb[:, 2 * jc: 2 * jc + 2], ps_t[:, 1:3])
        nc.scalar.activation(v_sb[:, :], d_sb[:, :], AF.Exp)

        # ---------------- S broadcast (via DRAM) + exp tables ----------------
        nc.sync.dma_start(s_dram[:], sdT_sb[0:1, :])
        S_b = const_pool.tile([128, N], F32, tag="Sb")
        nc.sync.dma_start(S_b[:, :], s_dram[None, :].broadcast_to((128, N)))
        U_b = const_pool.tile([128, N], BF16, tag="Ub")
        U2_b = const_pool.tile([128, N], BF16, tag="U2b")
        nc.scalar.activation(U_b[:, :], S_b[:, :], AF.Exp)
        nc.scalar.activation(U2_b[:, :], S_b[:, :], AF.Exp, scale=NEG)

        # ---------------- h_prime (V, bf16) ----------------
        V_sb = const_pool.tile([128, NCH * VW], BF16, tag="V")
        nc.vector.memset(V_sb[:, :], 1.0)
        for jc in range(NCH):
            ps_v = psw_pool.tile([128, F_OUT], F32, tag="psv")
            for fc in range(FC):
                nc.tensor.matmul(
                    ps_v[:, :],
                    hThi_sb[:, fc * N + jc * 128: fc * N + (jc + 1) * 128],
                    w_sb[:, fc * F_OUT:(fc + 1) * F_OUT],
                    start=(fc == 0), stop=(fc == FC - 1),
                )
            nc.vector.tensor_tensor(V_sb[:, jc * VW: jc * VW + F_OUT], ps_v[:, :],
                                    b_sb[:, :], op=ALU.add)

        # ---------------- attention j-loop ----------------
        pre_ctx.close()  # release pre-phase PSUM banks + h_T staging
        psbig_pool = ctx.enter_context(tc.tile_pool(name="psbig", bufs=1, space="PSUM"))
        ps_O = psbig_pool.tile([VW, N], F32, tag="psBIG")
        for jc in range(NCH):
            m_t = mask_pool.tile([128, N], BF16, tag="mt")
            nc.gpsimd.dma_start(m_t[:, :], maskT[jc * 128:(jc + 1) * 128, :])
            p_t = p_pool.tile([128, N], BF16, tag="pt")
            if routes[jc] == "exp":
                if use_prelu:
                    t_t = t_pool.tile([128, N], F32, tag="tt")
                    nc.scalar.activation(t_t[:, :], S_b[:, :], AF.Prelu,
                                         bias=d_sb[:, 2 * jc:2 * jc + 1], alpha=NEG)
                    e1_t = e_pool.tile([128, N], BF16, tag="et")
                    nc.scalar.activation(e1_t[:, :], t_t[:, :], AF.Exp)
                    nc.vector.tensor_tensor(p_t[:, :], e1_t[:, :], m_t[:, :], op=ALU.mult)
                else:
                    e1_t = t_pool.tile([128, N], BF16, tag="tt")
                    nc.scalar.activation(e1_t[:, :], S_b[:, :], AF.Exp,
                                         bias=d_sb[:, 2 * jc:2 * jc + 1])
                    e2_t = e_pool.tile([128, N], BF16, tag="et")
                    nc.scalar.activation(e2_t[:, :], S_b[:, :], AF.Exp, scale=NEG,
                                         bias=d_sb[:, 2 * jc + 1:2 * jc + 2])
                    c_t = c_pool.tile([128, N], BF16, tag="ct")
                    nc.vector.tensor_tensor(c_t[:, :], e1_t[:, :], e2_t[:, :], op=ALU.max)
                    nc.vector.tensor_tensor(p_t[:, :], c_t[:, :], m_t[:, :], op=ALU.mult)
            else:
                a_t = e_pool.tile([128, N], BF16, tag="et")
                nc.vector.tensor_scalar(a_t[:, :], U_b[:, :],
                                        v_sb[:, 2 * jc:2 * jc + 1], None, op0=ALU.mult)
                b_t = c_pool.tile([128, N], BF16, tag="ct")
                nc.vector.tensor_scalar(b_t[:, :], U2_b[:, :],
                                        v_sb[:, 2 * jc + 1:2 * jc + 2], None, op0=ALU.mult)
                c_t = t_pool.tile([128, N], BF16, tag="tt2")
                nc.vector.tensor_tensor(c_t[:, :], a_t[:, :], b_t[:, :], op=ALU.max)
                nc.vector.tensor_tensor(p_t[:, :], c_t[:, :], m_t[:, :], op=ALU.mult)
            for k in range(NSL):
                nc.tensor.matmul(ps_O[:, k * 512:(k + 1) * 512],
                                 V_sb[:, jc * VW:(jc + 1) * VW],
                                 p_t[:, k * 512:(k + 1) * 512],
                                 start=(jc == 0), stop=(jc == NCH - 1))

        # ---------------- tail: normalize + bias (no PE) ----------------
        tail_pool = ctx.enter_context(tc.tile_pool(name="tail", bufs=1))
        O_sb = tail_pool.tile([VW, N], F32, tag="Osb")
        nc.vector.tensor_copy(O_sb[:, :], ps_O[:, :])
        nc.sync.dma_start(sv_dram[:], O_sb[F_OUT:VW, :])
        sres = tail_pool.tile([128, N // 128], F32, tag="sres")
        nc.sync.dma_start(sres[:, :], sv_dram.rearrange("(p q) -> p q", p=128))
        rres = tail_pool.tile([128, N // 128], F32, tag="rres")
        nc.vector.reciprocal(rres[:, :], sres[:, :])
        nc.sync.dma_start(r_dram.rearrange("(p q) -> p q", p=128), rres[:, :])
        R_sb = tail_pool.tile([F_OUT, N], F32, tag="Rsb")
        nc.sync.dma_start(R_sb[:, :], r_dram[None, :].broadcast_to((F_OUT, N)))
        F_sb = tail_pool.tile([F_OUT, N], F32, tag="Fsb")
        nc.vector.tensor_tensor(F_sb[:, :], O_sb[0:F_OUT, :], R_sb[:, :], op=ALU.mult)
        nc.sync.dma_start(outT[:, :], F_sb[:, :])
    nc.compile()
    return nc


_CACHED_NC = None


def _get_nc():
    global _CACHED_NC
    if _CACHED_NC is None:
        _CACHED_NC = build_program()
    return _CACHED_NC


def _split_hilo(x):
    hi = x.astype(ml_dtypes.bfloat16)
    lo = (x - hi.astype(np.float32)).astype(ml_dtypes.bfloat16)
    return hi, lo


def _prep_inputs(h, adj, w, a_src, a_dst, b):
    h = np.asarray(h, dtype=np.float32)
    adj = np.asarray(adj)
    w = np.asarray(w, dtype=np.float32)
    a_src = np.asarray(a_src, dtype=np.float32)
    a_dst = np.asarray(a_dst, dtype=np.float32)
    b = np.asarray(b, dtype=np.float32)

    h_T = np.ascontiguousarray(h.T)
    hT_hi, hT_lo = _split_hilo(h_T)
    maskT = np.ascontiguousarray(adj.T).astype(ml_dtypes.bfloat16)
    b_bc = np.ascontiguousarray(np.broadcast_to(b[None, :], (128, F_OUT)),
                                dtype=np.float32)
    eye = np.eye(3, dtype=np.float32)

    in_maps = []
    for c in range(N_HEAD):
        wa_src = (w[c] @ a_src[c])[:, 0]              # [F_IN]
        wa_dst = (w[c] @ a_dst[c])[:, 0]
        cols = [wa_src, wa_dst, NEG * wa_dst]
        his, los = [], []
        for q in cols:
            qh, ql = _split_hilo(q.astype(np.float32))
            his.append(qh)
            los.append(ql)
        wa_hi3 = np.stack([x.astype(np.float32) for x in his], axis=1).astype(ml_dtypes.bfloat16)
        wa_lo3 = np.stack([x.astype(np.float32) for x in los], axis=1).astype(ml_dtypes.bfloat16)
        in_maps.append({
            "hT_hi": hT_hi,
            "hT_lo": hT_lo,
            "w_bf": np.ascontiguousarray(w[c]).astype(ml_dtypes.bfloat16),
            "wa_hi3": np.ascontiguousarray(wa_hi3),
            "wa_lo3": np.ascontiguousarray(wa_lo3),
            "eye3": eye,
            "maskT": maskT,
            "bvec": b_bc,
        })
    return in_maps


def _run(in_maps, trace=False, **kwargs):
    nc = _get_nc()
    return run_bass_kernel_spmd(nc, in_maps, list(range(N_HEAD)), trace=trace, **kwargs)


def kernel(h, adj, w, a_src, a_dst, b):
    in_maps = _prep_inputs(h, adj, w, a_src, a_dst, b)
    res = _run(in_maps)
    out = np.stack([np.ascontiguousarray(res.results[c]["outT"].T)
                    for c in range(N_HEAD)])
    return out.astype(np.float32)


# revision 16
# speedup vs baseline: 1.2116x; 1.0773x over previous
"""Trainium2 Bass kernel for nn_MultiHeadGraphAttention.

Multi-head graph attention (GAT-style):
    h_prime = einsum('nf,hfo->hno', h, w)
    attn    = softmax(where(adj, leakyrelu(s_i + d_j), -inf), axis=-1)
    out     = attn @ h_prime + b
with s = h_prime @ a_src, d = h_prime @ a_dst, n=4096, H=8, f_out=64.

Sharding: one head per NeuronCore (8 cores). Inside a core everything is
computed in a transposed [j, i] layout so the attention matrix feeds the
PE matmul directly as the moving operand with contraction over j:

  P^T[j, i] = adj[i, j] * exp(leakyrelu(s_i + d_j))
  out^T[o, i] = sum_j Vtilde[j, o] * P^T[j, i]     (Vtilde = [h_prime | 1])

The ones column of Vtilde yields the softmax denominators for free.
exp(leakyrelu(x)) == max(exp(x), exp(0.2 x)) (exp is monotonic), giving
two per-chunk strategies balanced across ScalarE and VectorE:
  - "exp"  : ACT Prelu(alpha=0.2) + ACT Exp (bias = d_j per-partition),
             then one DVE mask multiply
  - "rank1": exp(s_i + d_j) = u_i * v_j is rank-1; DVE-only via
             tensor_scalar on precomputed u = exp(s) broadcast tiles and
             v = exp(d) per-partition columns, then max + mask.

fp32 matmuls run at half rate on TRN2's PE, so the h_prime/s/d
projections use bf16 hi+lo split-compensation (exact to ~fp32 for s/d:
bf16 products are exact in fp32, and the h*wa projections accumulate
hi*hi + hi*lo + lo*hi (+lo*lo) in fp32 PSUM).

Host-side prep (sharding/layout only): h transposed + split hi/lo bf16,
adj transposed and cast to bf16 {0,1}, per-head weight columns, final
out^T -> out transpose.
"""
import sys

if "/opt/trn_rl_repo" not in sys.path:
    sys.path.insert(0, "/opt/trn_rl_repo")

from contextlib import ExitStack

import ml_dtypes
import numpy as np

import concourse.bass as bass
import concourse.bacc as bacc
import concourse.tile as tile
from concourse import mybir
from concourse.bass_utils import run_bass_kernel_spmd

F32 = mybir.dt.float32
BF16 = mybir.dt.bfloat16
AF = mybir.ActivationFunctionType
ALU = mybir.AluOpType

N = 4096
F_IN = 256
N_HEAD = 8
F_OUT = 64
NEG = 0.2
NCH = N // 128        # 32 j-chunks
NSL = N // 512        # 8 512-slices
FC = F_IN // 128      # 2 f-chunks
VW = F_OUT + 1        # 65: V columns + ones column

# Per-chunk route: "exp" (2x ACT + 1x DVE) or "rank1" (4x DVE).
# exp-heavy at the start (rank1 needs the exp(d)/exp(s) tables).
_R1 = {2, 5, 8, 10, 13, 16, 18, 21, 24, 26, 29, 31}
ROUTES = ["rank1" if i in _R1 else "exp" for i in range(NCH)]
assert len(ROUTES) == NCH and sum(r == "exp" for r in ROUTES) == 20


def build_program(routes=ROUTES, use_prelu=True):
    nc = bacc.Bacc("TRN2", target_bir_lowering=False, debug=False)
    hT_hi = nc.dram_tensor("hT_hi", [F_IN, N], BF16, kind="ExternalInput").ap()
    hT_lo = nc.dram_tensor("hT_lo", [F_IN, N], BF16, kind="ExternalInput").ap()
    w_bf = nc.dram_tensor("w_bf", [F_IN, F_OUT], BF16, kind="ExternalInput").ap()
    wa_hi3 = nc.dram_tensor("wa_hi3", [F_IN, 3], BF16, kind="ExternalInput").ap()
    wa_lo3 = nc.dram_tensor("wa_lo3", [F_IN, 3], BF16, kind="ExternalInput").ap()
    eye3 = nc.dram_tensor("eye3", [3, 3], F32, kind="ExternalInput").ap()
    maskT = nc.dram_tensor("maskT", [N, N], BF16, kind="ExternalInput").ap()
    bvec = nc.dram_tensor("bvec", [128, F_OUT], F32, kind="ExternalInput").ap()
    outT = nc.dram_tensor("outT", [F_OUT, N], F32, kind="ExternalOutput").ap()
    s_dram = nc.dram_tensor("s_scratch", [N], F32).ap()
    sv_dram = nc.dram_tensor("sv_scratch", [N], F32).ap()
    r_dram = nc.dram_tensor("r_scratch", [N], F32).ap()

    with tile.TileContext(nc) as tc, ExitStack() as ctx:
        const_pool = ctx.enter_context(tc.tile_pool(name="const", bufs=1))
        p_pool = ctx.enter_context(tc.tile_pool(name="pw", bufs=3))
        t_pool = ctx.enter_context(tc.tile_pool(name="tw", bufs=2))
        e_pool = ctx.enter_context(tc.tile_pool(name="ew", bufs=2))
        c_pool = ctx.enter_context(tc.tile_pool(name="cw", bufs=2))
        mask_pool = ctx.enter_context(tc.tile_pool(name="mask", bufs=2))
        pre_ctx = ExitStack()
        psw_pool = pre_ctx.enter_context(tc.tile_pool(name="psw", bufs=2, space="PSUM"))
        pre_pool = pre_ctx.enter_context(tc.tile_pool(name="pre", bufs=1))

        # ---------------- constant loads ----------------
        hThi_sb = pre_pool.tile([128, FC * N], BF16, tag="hThi")
        hTlo_sb = pre_pool.tile([128, FC * N], BF16, tag="hTlo")
        H2 = N // 2
        for half in range(2):
            for fc in range(FC):
                nc.sync.dma_start(
                    hThi_sb[:, fc * N + half * H2: fc * N + (half + 1) * H2],
                    hT_hi[fc * 128:(fc + 1) * 128, half * H2:(half + 1) * H2])
            for fc in range(FC):
                nc.sync.dma_start(
                    hTlo_sb[:, fc * N + half * H2: fc * N + (half + 1) * H2],
                    hT_lo[fc * 128:(fc + 1) * 128, half * H2:(half + 1) * H2])
        wahi_sb = const_pool.tile([128, FC * 3], BF16, tag="wahi")
        walo_sb = const_pool.tile([128, FC * 3], BF16, tag="walo")
        for fc in range(FC):
            nc.sync.dma_start(wahi_sb[:, fc * 3:(fc + 1) * 3], wa_hi3[fc * 128:(fc + 1) * 128, :])
            nc.sync.dma_start(walo_sb[:, fc * 3:(fc + 1) * 3], wa_lo3[fc * 128:(fc + 1) * 128, :])
        w_sb = const_pool.tile([128, FC * F_OUT], BF16, tag="w")
        for fc in range(FC):
            nc.sync.dma_start(w_sb[:, fc * F_OUT:(fc + 1) * F_OUT],
                              w_bf[fc * 128:(fc + 1) * 128, :])
        eye_sb = const_pool.tile([3, 3], F32, tag="eye")
        nc.sync.dma_start(eye_sb[:, :], eye3[:, :])
        b_sb = const_pool.tile([128, F_OUT], F32, tag="b")
        nc.sync.dma_start(b_sb[:, :], bvec[:, :])

        # ---------------- s^T/d^T rows (hi/lo compensated) ----------------
        # lhsT = wa6 [f, 6] = [s_hi d_hi d5_hi s_lo d_lo d5_lo] coefficient
        # columns; accumulating over {hi,lo} x {f-chunks} of h gives rows
        # 0:3 = (h_hi+h_lo)*hi-coef... true rows = row[k] + row[k+3].
        sdT_sb = pre_pool.tile([3, N], F32, tag="sdT")
        for sl in range(NSL):
            ps_sd = psw_pool.tile([3, 512], F32, tag="pssd")
            combos = [(wa, hsb) for fc in range(FC)
                      for wa in (wahi_sb, walo_sb) for hsb in (hThi_sb, hTlo_sb)]
            ncomb = len(combos)
            for ci, (wa, hsb) in enumerate(combos):
                fc = ci // 4
                nc.tensor.matmul(ps_sd[:, :], wa[:, fc * 3:(fc + 1) * 3],
                                 hsb[:, fc * N + sl * 512: fc * N + (sl + 1) * 512],
                                 start=(ci == 0), stop=(ci == ncomb - 1))
            nc.vector.tensor_copy(sdT_sb[0:3, sl * 512:(sl + 1) * 512], ps_sd[:, :])

        # ---------------- d columns via PE transpose ----------------
        d_sb = const_pool.tile([128, 2 * NCH], F32, tag="d")
        v_sb = const_pool.tile([128, 2 * NCH], F32, tag="v")
        for jc in range(NCH):
            ps_t = psw_pool.tile([128, 3], F32, tag="pst")
            nc.tensor.transpose(ps_t[:, :], sdT_sb[0:3, jc * 128:(jc + 1) * 128],
                                eye_sb[:, :])
            nc.vector.tensor_copy(d_sb[:, 2 * jc: 2 * jc + 2], ps_t[:, 1:3])
        nc.scalar.activation(v_sb[:, :], d_sb[:, :], AF.Exp)

        # ---------------- S broadcast (via DRAM) + exp tables ----------------
        nc.sync.dma_start(s_dram[:], sdT_sb[0:1, :])
        S_b = const_pool.tile([128, N], F32, tag="Sb")
        nc.sync.dma_start(S_b[:, :], s_dram[None, :].broadcast_to((128, N)))
        U_b = const_pool.tile([128, N], BF16, tag="Ub")
        U2_b = const_pool.tile([128, N], BF16, tag="U2b")
        nc.scalar.activation(U_b[:, :], S_b[:, :], AF.Exp)
        nc.scalar.activation(U2_b[:, :], S_b[:, :], AF.Exp, scale=NEG)

        # ---------------- h_prime (V, bf16) ----------------
        V_sb = const_pool.tile([128, NCH * VW], BF16, tag="V")
        nc.vector.memset(V_sb[:, :], 1.0)
        for jc in range(NCH):
            ps_v = psw_pool.tile([128, F_OUT], F32, tag="psv")
            for fc in range(FC):
                nc.tensor.matmul(
                    ps_v[:, :],
                    hThi_sb[:, fc * N + jc * 128: fc * N + (jc + 1) * 128],
                    w_sb[:, fc * F_OUT:(fc + 1) * F_OUT],
                    start=(fc == 0), stop=(fc == FC - 1),
                )
            nc.vector.tensor_tensor(V_sb[:, jc * VW: jc * VW + F_OUT], ps_v[:, :],
                                    b_sb[:, :], op=ALU.add)

        # ---------------- attention j-loop ----------------
        pre_ctx.close()  # release pre-phase PSUM banks + h_T staging
        psbig_pool = ctx.enter_context(tc.tile_pool(name="psbig", bufs=1, space="PSUM"))
        ps_O = psbig_pool.tile([VW, N], F32, tag="psBIG")
        for jc in range(NCH):
            m_t = mask_pool.tile([128, N], BF16, tag="mt")
            nc.sync.dma_start(m_t[:, :], maskT[jc * 128:(jc + 1) * 128, :])
            p_t = p_pool.tile([128, N], BF16, tag="pt")
            if routes[jc] == "exp":
                if use_prelu:
                    t_t = t_pool.tile([128, N], F32, tag="tt")
                    nc.scalar.activation(t_t[:, :], S_b[:, :], AF.Prelu,
                                         bias=d_sb[:, 2 * jc:2 * jc + 1], alpha=NEG)
                    e1_t = e_pool.tile([128, N], BF16, tag="et")
                    nc.scalar.activation(e1_t[:, :], t_t[:, :], AF.Exp)
                    nc.vector.tensor_tensor(p_t[:, :], e1_t[:, :], m_t[:, :], op=ALU.mult)
                else:
                    e1_t = t_pool.tile([128, N], BF16, tag="tt")
                    nc.scalar.activation(e1_t[:, :], S_b[:, :], AF.Exp,
                                         bias=d_sb[:, 2 * jc:2 * jc + 1])
                    e2_t = e_pool.tile([128, N], BF16, tag="et")
                    nc.scalar.activation(e2_t[:, :], S_b[:, :], AF.Exp, scale=NEG,
                                         bias=d_sb[:, 2 * jc + 1:2 * jc + 2])
                    c_t = c_pool.tile([128, N], BF16, tag="ct")
                    nc.vector.tensor_tensor(c_t[:, :], e1_t[:, :], e2_t[:, :], op=ALU.max)
                    nc.vector.tensor_tensor(p_t[:, :], c_t[:, :], m_t[:, :], op=ALU.mult)
            else:
                a_t = e_pool.tile([128, N], BF16, tag="et")
                nc.vector.tensor_scalar(a_t[:, :], U_b[:, :],
                                        v_sb[:, 2 * jc:2 * jc + 1], None, op0=ALU.mult)
                b_t = c_pool.tile([128, N], BF16, tag="ct")
                nc.vector.tensor_scalar(b_t[:, :], U2_b[:, :],
                                        v_sb[:, 2 * jc + 1:2 * jc + 2], None, op0=ALU.mult)
                c_t = t_pool.tile([128, N], BF16, tag="tt2")
                nc.vector.tensor_tensor(c_t[:, :], a_t[:, :], b_t[:, :], op=ALU.max)
                nc.vector.tensor_tensor(p_t[:, :], c_t[:, :], m_t[:, :], op=ALU.mult)
            for k in range(NSL):
                nc.tensor.matmul(ps_O[:, k * 512:(k + 1) * 512],
                                 V_sb[:, jc * VW:(jc + 1) * VW],
                                 p_t[:, k * 512:(k + 1) * 512],
                                 start=(jc == 0), stop=(jc == NCH - 1))

        # ---------------- tail: normalize + bias (no PE) ----------------
        tail_pool = ctx.enter_context(tc.tile_pool(name="tail", bufs=1))
        O_sb = tail_pool.tile([VW, N], F32, tag="Osb")
        nc.vector.tensor_copy(O_sb[:, :], ps_O[:, :])
        nc.sync.dma_start(sv_dram[:], O_sb[F_OUT:VW, :])
        sres = tail_pool.tile([128, N // 128], F32, tag="sres")
        nc.sync.dma_start(sres[:, :], sv_dram.rearrange("(p q) -> p q", p=128))
        rres = tail_pool.tile([128, N // 128], F32, tag="rres")
        nc.vector.reciprocal(rres[:, :], sres[:, :])
        nc.sync.dma_start(r_dram.rearrange("(p q) -> p q", p=128), rres[:, :])
        R_sb = tail_pool.tile([F_OUT, N], F32, tag="Rsb")
        nc.sync.dma_start(R_sb[:, :], r_dram[None, :].broadcast_to((F_OUT, N)))
        F_sb = tail_pool.tile([F_OUT, N], F32, tag="Fsb")
        nc.vector.tensor_tensor(F_sb[:, :], O_sb[0:F_OUT, :], R_sb[:, :], op=ALU.mult)
        nc.sync.dma_start(outT[:, :], F_sb[:, :])
    nc.compile()
    return nc


_CACHED_NC = None


def _get_nc():
    global _CACHED_NC
    if _CACHED_NC is None:
        _CACHED_NC = build_program()
    return _CACHED_NC


def _split_hilo(x):
    hi = x.astype(ml_dtypes.bfloat16)
    lo = (x - hi.astype(np.float32)).astype(ml_dtypes.bfloat16)
    return hi, lo


def _prep_inputs(h, adj, w, a_src, a_dst, b):
    h = np.asarray(h, dtype=np.float32)
    adj = np.asarray(adj)
    w = np.asarray(w, dtype=np.float32)
    a_src = np.asarray(a_src, dtype=np.float32)
    a_dst = np.asarray(a_dst, dtype=np.float32)
    b = np.asarray(b, dtype=np.float32)

    h_T = np.ascontiguousarray(h.T)
    hT_hi, hT_lo = _split_hilo(h_T)
    maskT = np.ascontiguousarray(adj.T).astype(ml_dtypes.bfloat16)
    b_bc = np.ascontiguousarray(np.broadcast_to(b[None, :], (128, F_OUT)),
                                dtype=np.float32)
    eye = np.eye(3, dtype=np.float32)

    in_maps = []
    for c in range(N_HEAD):
        wa_src = (w[c] @ a_src[c])[:, 0]              # [F_IN]
        wa_dst = (w[c] @ a_dst[c])[:, 0]
        cols = [wa_src, wa_dst, NEG * wa_dst]
        his, los = [], []
        for q in cols:
            qh, ql = _split_hilo(q.astype(np.float32))
            his.append(qh)
            los.append(ql)
        wa_hi3 = np.stack([x.astype(np.float32) for x in his], axis=1).astype(ml_dtypes.bfloat16)
        wa_lo3 = np.stack([x.astype(np.float32) for x in los], axis=1).astype(ml_dtypes.bfloat16)
        in_maps.append({
            "hT_hi": hT_hi,
            "hT_lo": hT_lo,
            "w_bf": np.ascontiguousarray(w[c]).astype(ml_dtypes.bfloat16),
            "wa_hi3": np.ascontiguousarray(wa_hi3),
            "wa_lo3": np.ascontiguousarray(wa_lo3),
            "eye3": eye,
            "maskT": maskT,
            "bvec": b_bc,
        })
    return in_maps


def _run(in_maps, trace=False, **kwargs):
    nc = _get_nc()
    return run_bass_kernel_spmd(nc, in_maps, list(range(N_HEAD)), trace=trace, **kwargs)


def kernel(h, adj, w, a_src, a_dst, b):
    in_maps = _prep_inputs(h, adj, w, a_src, a_dst, b)
    res = _run(in_maps)
    out = np.stack([np.ascontiguousarray(res.results[c]["outT"].T)
                    for c in range(N_HEAD)])
    return out.astype(np.float32)


# revision 17
# speedup vs baseline: 1.2662x; 1.0450x over previous
"""Trainium2 Bass kernel for nn_MultiHeadGraphAttention.

Multi-head graph attention (GAT-style):
    h_prime = einsum('nf,hfo->hno', h, w)
    attn    = softmax(where(adj, leakyrelu(s_i + d_j), -inf), axis=-1)
    out     = attn @ h_prime + b
with s = h_prime @ a_src, d = h_prime @ a_dst, n=4096, H=8, f_out=64.

Sharding: one head per NeuronCore (8 cores). Inside a core everything is
computed in a transposed [j, i] layout so the attention matrix feeds the
PE matmul directly as the moving operand with contraction over j:

  P^T[j, i] = adj[i, j] * exp(leakyrelu(s_i + d_j))
  out^T[o, i] = sum_j Vtilde[j, o] * P^T[j, i]     (Vtilde = [h_prime | 1])

The ones column of Vtilde yields the softmax denominators for free.
exp(leakyrelu(x)) == max(exp(x), exp(0.2 x)) (exp is monotonic), giving
two per-chunk strategies balanced across ScalarE and VectorE:
  - "exp"  : ACT Prelu(alpha=0.2) + ACT Exp (bias = d_j per-partition),
             then one DVE mask multiply
  - "rank1": exp(s_i + d_j) = u_i * v_j is rank-1; DVE-only via
             tensor_scalar on precomputed u = exp(s) broadcast tiles and
             v = exp(d) per-partition columns, then max + mask.

fp32 matmuls run at half rate on TRN2's PE, so the h_prime/s/d
projections use bf16 hi+lo split-compensation (exact to ~fp32 for s/d:
bf16 products are exact in fp32, and the h*wa projections accumulate
hi*hi + hi*lo + lo*hi (+lo*lo) in fp32 PSUM).

Host-side prep (sharding/layout only): h transposed + split hi/lo bf16,
adj transposed and cast to bf16 {0,1}, per-head weight columns, final
out^T -> out transpose.
"""
import sys

if "/opt/trn_rl_repo" not in sys.path:
    sys.path.insert(0, "/opt/trn_rl_repo")

from contextlib import ExitStack

import ml_dtypes
import numpy as np

import concourse.bass as bass
import concourse.bacc as bacc
import concourse.tile as tile
from concourse import mybir
from concourse.bass_utils import run_bass_kernel_spmd

F32 = mybir.dt.float32
BF16 = mybir.dt.bfloat16
AF = mybir.ActivationFunctionType
ALU = mybir.AluOpType

N = 4096
F_IN = 256
N_HEAD = 8
F_OUT = 64
NEG = 0.2
NCH = N // 128        # 32 j-chunks
NSL = N // 512        # 8 512-slices
FC = F_IN // 128      # 2 f-chunks
VW = F_OUT + 1        # 65: V columns + ones column

# Per-chunk route: "exp" (2x ACT + 1x DVE) or "rank1" (4x DVE).
# exp-heavy at the start (rank1 needs the exp(d)/exp(s) tables).
_R1 = {2, 5, 8, 10, 13, 16, 18, 21, 24, 26, 29, 31}
ROUTES = ["rank1" if i in _R1 else "exp" for i in range(NCH)]
assert len(ROUTES) == NCH and sum(r == "exp" for r in ROUTES) == 20


def build_program(routes=ROUTES, use_prelu=True):
    nc = bacc.Bacc("TRN2", target_bir_lowering=False, debug=False)
    hT_hi = nc.dram_tensor("hT_hi", [F_IN, N], BF16, kind="ExternalInput").ap()
    hT_lo = nc.dram_tensor("hT_lo", [F_IN, N], BF16, kind="ExternalInput").ap()
    w_bf = nc.dram_tensor("w_bf", [F_IN, F_OUT], BF16, kind="ExternalInput").ap()
    wa_hi3 = nc.dram_tensor("wa_hi3", [F_IN, 3], BF16, kind="ExternalInput").ap()
    wa_lo3 = nc.dram_tensor("wa_lo3", [F_IN, 3], BF16, kind="ExternalInput").ap()
    eye3 = nc.dram_tensor("eye3", [3, 3], F32, kind="ExternalInput").ap()
    maskT = nc.dram_tensor("maskT", [N, N], BF16, kind="ExternalInput").ap()
    bvec = nc.dram_tensor("bvec", [128, F_OUT], F32, kind="ExternalInput").ap()
    outT = nc.dram_tensor("outT", [F_OUT, N], F32, kind="ExternalOutput").ap()
    s_dram = nc.dram_tensor("s_scratch", [N], F32).ap()
    sv_dram = nc.dram_tensor("sv_scratch", [N], F32).ap()
    r_dram = nc.dram_tensor("r_scratch", [N], F32).ap()

    with tile.TileContext(nc) as tc, ExitStack() as ctx:
        const_pool = ctx.enter_context(tc.tile_pool(name="const", bufs=1))
        mask_pool = ctx.enter_context(tc.tile_pool(name="mask", bufs=3))
        pre_ctx = ExitStack()
        psw_pool = pre_ctx.enter_context(tc.tile_pool(name="psw", bufs=2, space="PSUM"))
        pre_pool = pre_ctx.enter_context(tc.tile_pool(name="pre", bufs=1))

        # ---------------- constant loads ----------------
        hThi_sb = pre_pool.tile([128, FC * N], BF16, tag="hThi")
        hTlo_sb = pre_pool.tile([128, FC * N], BF16, tag="hTlo")
        H2 = N // 2
        for half in range(2):
            for fc in range(FC):
                nc.sync.dma_start(
                    hThi_sb[:, fc * N + half * H2: fc * N + (half + 1) * H2],
                    hT_hi[fc * 128:(fc + 1) * 128, half * H2:(half + 1) * H2])
            for fc in range(FC):
                nc.sync.dma_start(
                    hTlo_sb[:, fc * N + half * H2: fc * N + (half + 1) * H2],
                    hT_lo[fc * 128:(fc + 1) * 128, half * H2:(half + 1) * H2])
        wahi_sb = const_pool.tile([128, FC * 3], BF16, tag="wahi")
        walo_sb = const_pool.tile([128, FC * 3], BF16, tag="walo")
        for fc in range(FC):
            nc.sync.dma_start(wahi_sb[:, fc * 3:(fc + 1) * 3], wa_hi3[fc * 128:(fc + 1) * 128, :])
            nc.sync.dma_start(walo_sb[:, fc * 3:(fc + 1) * 3], wa_lo3[fc * 128:(fc + 1) * 128, :])
        w_sb = const_pool.tile([128, FC * F_OUT], BF16, tag="w")
        for fc in range(FC):
            nc.sync.dma_start(w_sb[:, fc * F_OUT:(fc + 1) * F_OUT],
                              w_bf[fc * 128:(fc + 1) * 128, :])
        eye_sb = const_pool.tile([3, 3], F32, tag="eye")
        nc.sync.dma_start(eye_sb[:, :], eye3[:, :])
        b_sb = const_pool.tile([128, F_OUT], F32, tag="b")
        nc.sync.dma_start(b_sb[:, :], bvec[:, :])

        # ---------------- s^T/d^T rows (hi/lo compensated) ----------------
        # lhsT = wa6 [f, 6] = [s_hi d_hi d5_hi s_lo d_lo d5_lo] coefficient
        # columns; accumulating over {hi,lo} x {f-chunks} of h gives rows
        # 0:3 = (h_hi+h_lo)*hi-coef... true rows = row[k] + row[k+3].
        sdT_sb = pre_pool.tile([3, N], F32, tag="sdT")
        for sl in range(NSL):
            ps_sd = psw_pool.tile([3, 512], F32, tag="pssd")
            combos = [(wa, hsb) for fc in range(FC)
                      for wa in (wahi_sb, walo_sb) for hsb in (hThi_sb, hTlo_sb)
                      if not (wa is walo_sb and hsb is hTlo_sb)]
            ncomb = len(combos)
            for ci, (wa, hsb) in enumerate(combos):
                fc = ci // 3
                nc.tensor.matmul(ps_sd[:, :], wa[:, fc * 3:(fc + 1) * 3],
                                 hsb[:, fc * N + sl * 512: fc * N + (sl + 1) * 512],
                                 start=(ci == 0), stop=(ci == ncomb - 1))
            nc.vector.tensor_copy(sdT_sb[0:3, sl * 512:(sl + 1) * 512], ps_sd[:, :])

        # ---------------- d columns via PE transpose ----------------
        d_sb = const_pool.tile([128, 2 * NCH], F32, tag="d")
        v_sb = const_pool.tile([128, 2 * NCH], F32, tag="v")
        for jc in range(NCH):
            ps_t = psw_pool.tile([128, 3], F32, tag="pst")
            nc.tensor.transpose(ps_t[:, :], sdT_sb[0:3, jc * 128:(jc + 1) * 128],
                                eye_sb[:, :])
            nc.vector.tensor_copy(d_sb[:, 2 * jc: 2 * jc + 2], ps_t[:, 1:3])
        nc.scalar.activation(v_sb[:, :], d_sb[:, :], AF.Exp)

        # ---------------- S broadcast (via DRAM) + exp tables ----------------
        nc.sync.dma_start(s_dram[:], sdT_sb[0:1, :])
        S_b = const_pool.tile([128, N], F32, tag="Sb")
        nc.sync.dma_start(S_b[:, :], s_dram[None, :].broadcast_to((128, N)))
        U_b = const_pool.tile([128, N], BF16, tag="Ub")
        U2_b = const_pool.tile([128, N], BF16, tag="U2b")
        nc.scalar.activation(U_b[:, :], S_b[:, :], AF.Exp)
        nc.scalar.activation(U2_b[:, :], S_b[:, :], AF.Exp, scale=NEG)

        # ---------------- h_prime (V, bf16) ----------------
        V_sb = const_pool.tile([128, NCH * VW], BF16, tag="V")
        nc.vector.memset(V_sb[:, :], 1.0)
        for jc in range(NCH):
            ps_v = psw_pool.tile([128, F_OUT], F32, tag="psv")
            for fc in range(FC):
                nc.tensor.matmul(
                    ps_v[:, :],
                    hThi_sb[:, fc * N + jc * 128: fc * N + (jc + 1) * 128],
                    w_sb[:, fc * F_OUT:(fc + 1) * F_OUT],
                    start=(fc == 0), stop=(fc == FC - 1),
                )
            nc.vector.tensor_tensor(V_sb[:, jc * VW: jc * VW + F_OUT], ps_v[:, :],
                                    b_sb[:, :], op=ALU.add)

        # ---------------- attention j-loop ----------------
        pre_ctx.close()  # release pre-phase PSUM banks + h_T staging
        p_pool = ctx.enter_context(tc.tile_pool(name="pw", bufs=3))
        t_pool = ctx.enter_context(tc.tile_pool(name="tw", bufs=3))
        e_pool = ctx.enter_context(tc.tile_pool(name="ew", bufs=2))
        c_pool = ctx.enter_context(tc.tile_pool(name="cw", bufs=2))
        psbig_pool = ctx.enter_context(tc.tile_pool(name="psbig", bufs=1, space="PSUM"))
        ps_O = psbig_pool.tile([VW, N], F32, tag="psBIG")
        for jc in range(NCH):
            m_t = mask_pool.tile([128, N], BF16, tag="mt")
            nc.sync.dma_start(m_t[:, :], maskT[jc * 128:(jc + 1) * 128, :])
            p_t = p_pool.tile([128, N], BF16, tag="pt")
            if routes[jc] == "exp":
                if use_prelu:
                    t_t = t_pool.tile([128, N], F32, tag="tt")
                    nc.scalar.activation(t_t[:, :], S_b[:, :], AF.Prelu,
                                         bias=d_sb[:, 2 * jc:2 * jc + 1], alpha=NEG)
                    e1_t = e_pool.tile([128, N], BF16, tag="et")
                    nc.scalar.activation(e1_t[:, :], t_t[:, :], AF.Exp)
                    nc.vector.tensor_tensor(p_t[:, :], e1_t[:, :], m_t[:, :], op=ALU.mult)
                else:
                    e1_t = t_pool.tile([128, N], BF16, tag="tt")
                    nc.scalar.activation(e1_t[:, :], S_b[:, :], AF.Exp,
                                         bias=d_sb[:, 2 * jc:2 * jc + 1])
                    e2_t = e_pool.tile([128, N], BF16, tag="et")
                    nc.scalar.activation(e2_t[:, :], S_b[:, :], AF.Exp, scale=NEG,
                                         bias=d_sb[:, 2 * jc + 1:2 * jc + 2])
                    c_t = c_pool.tile([128, N], BF16, tag="ct")
                    nc.vector.tensor_tensor(c_t[:, :], e1_t[:, :], e2_t[:, :], op=ALU.max)
                    nc.vector.tensor_tensor(p_t[:, :], c_t[:, :], m_t[:, :], op=ALU.mult)
            else:
                a_t = e_pool.tile([128, N], BF16, tag="et")
                nc.vector.tensor_scalar(a_t[:, :], U_b[:, :],
                                        v_sb[:, 2 * jc:2 * jc + 1], None, op0=ALU.mult)
                b_t = c_pool.tile([128, N], BF16, tag="ct")
                nc.vector.tensor_scalar(b_t[:, :], U2_b[:, :],
                                        v_sb[:, 2 * jc + 1:2 * jc + 2], None, op0=ALU.mult)
                c_t = t_pool.tile([128, N], BF16, tag="tt2")
                nc.vector.tensor_tensor(c_t[:, :], a_t[:, :], b_t[:, :], op=ALU.max)
                nc.vector.tensor_tensor(p_t[:, :], c_t[:, :], m_t[:, :], op=ALU.mult)
            for k in range(NSL):
                nc.tensor.matmul(ps_O[:, k * 512:(k + 1) * 512],
                                 V_sb[:, jc * VW:(jc + 1) * VW],
                                 p_t[:, k * 512:(k + 1) * 512],
                                 start=(jc == 0), stop=(jc == NCH - 1))

        # ---------------- tail: normalize (no PE) ----------------
        tail_pool = ctx.enter_context(tc.tile_pool(name="tail", bufs=1))
        Srow = const_pool.tile([1, N], F32, tag="Sb")
        nc.vector.tensor_copy(Srow[:, :], ps_O[F_OUT:VW, :])
        nc.sync.dma_start(sv_dram[:], Srow[:, :])
        sres = tail_pool.tile([128, N // 128], F32, tag="sres")
        nc.sync.dma_start(sres[:, :], sv_dram.rearrange("(p q) -> p q", p=128))
        rres = tail_pool.tile([128, N // 128], F32, tag="rres")
        nc.vector.reciprocal(rres[:, :], sres[:, :])
        nc.sync.dma_start(r_dram.rearrange("(p q) -> p q", p=128), rres[:, :])
        R_sb = t_pool.tile([F_OUT, N], F32, tag="tt")
        nc.sync.dma_start(R_sb[:, :], r_dram[None, :].broadcast_to((F_OUT, N)))
        F_sb = t_pool.tile([F_OUT, N], F32, tag="tt")
        nc.vector.tensor_tensor(F_sb[:, :], ps_O[0:F_OUT, :], R_sb[:, :], op=ALU.mult)
        nc.sync.dma_start(outT[:, :], F_sb[:, :])
    nc.compile()
    return nc


_CACHED_NC = None


def _get_nc():
    global _CACHED_NC
    if _CACHED_NC is None:
        _CACHED_NC = build_program()
    return _CACHED_NC


def _split_hilo(x):
    hi = x.astype(ml_dtypes.bfloat16)
    lo = (x - hi.astype(np.float32)).astype(ml_dtypes.bfloat16)
    return hi, lo


def _prep_inputs(h, adj, w, a_src, a_dst, b):
    h = np.asarray(h, dtype=np.float32)
    adj = np.asarray(adj)
    w = np.asarray(w, dtype=np.float32)
    a_src = np.asarray(a_src, dtype=np.float32)
    a_dst = np.asarray(a_dst, dtype=np.float32)
    b = np.asarray(b, dtype=np.float32)

    h_T = np.ascontiguousarray(h.T)
    hT_hi, hT_lo = _split_hilo(h_T)
    maskT = np.ascontiguousarray(adj.T).astype(ml_dtypes.bfloat16)
    b_bc = np.ascontiguousarray(np.broadcast_to(b[None, :], (128, F_OUT)),
                                dtype=np.float32)
    eye = np.eye(3, dtype=np.float32)

    in_maps = []
    for c in range(N_HEAD):
        wa_src = (w[c] @ a_src[c])[:, 0]              # [F_IN]
        wa_dst = (w[c] @ a_dst[c])[:, 0]
        cols = [wa_src, wa_dst, NEG * wa_dst]
        his, los = [], []
        for q in cols:
            qh, ql = _split_hilo(q.astype(np.float32))
            his.append(qh)
            los.append(ql)
        wa_hi3 = np.stack([x.astype(np.float32) for x in his], axis=1).astype(ml_dtypes.bfloat16)
        wa_lo3 = np.stack([x.astype(np.float32) for x in los], axis=1).astype(ml_dtypes.bfloat16)
        in_maps.append({
            "hT_hi": hT_hi,
            "hT_lo": hT_lo,
            "w_bf": np.ascontiguousarray(w[c]).astype(ml_dtypes.bfloat16),
            "wa_hi3": np.ascontiguousarray(wa_hi3),
            "wa_lo3": np.ascontiguousarray(wa_lo3),
            "eye3": eye,
            "maskT": maskT,
            "bvec": b_bc,
        })
    return in_maps


def _run(in_maps, trace=False, **kwargs):
    nc = _get_nc()
    return run_bass_kernel_spmd(nc, in_maps, list(range(N_HEAD)), trace=trace, **kwargs)


def kernel(h, adj, w, a_src, a_dst, b):
    in_maps = _prep_inputs(h, adj, w, a_src, a_dst, b)
    res = _run(in_maps)
    out = np.stack([np.ascontiguousarray(res.results[c]["outT"].T)
                    for c in range(N_HEAD)])
    return out.astype(np.float32)
